# revision 11
# baseline (speedup 1.0000x reference)
"""
Binary Conv2d (BBCU-style) block on 8 Trainium2 NeuronCores.

Computation (per reference):
    z  = sign(x + move0_bias)                    # binarized activations in {-1,0,1}
    bw = scale[o] * sign(W)                      # binarized weights, per-out-channel scale
    y  = conv3x3(z, bw, pad=1)
    y  = prelu(y + pr_bias0, a) + pr_bias1 + x   # RPReLU + identity

Key exactness trick: the conv operands are exact small values (z in {-1,0,1},
sign(W) in {-1,0,1}) so we run the conv as fp8e4 matmuls with fp32 PSUM
accumulation — bit-exact integer counts (|sum| <= 576 << 2^24). The per-channel
`scale` folds into the epilogue affine constants.

Sharding: data-parallel over batch. 16 images / 8 cores = 2 images per core.

Per-core layout ("parity layout"): partitions = 64 channels x row-parity
(parts 0-63 even rows, 64-127 odd rows). SBUF tensors:
  xt   [128, P*256] f32   : chunk of G=2P rows of x (per-chunk, identity + sign input)
  zs1  [128, 130*272] fp8 : sign values, one 272B slot per row-pair index;
                            byte j in a slot = col j-1 (byte 0 / 257 = zero pad)
  zs2  [128, 130*272] fp8 : copy of zs1 with the odd block shifted +2 slots,
                            so the "cross-pair" matmuls read an aligned AP.
Conv = 6 matmuls per PSUM tile [128,512] (2 row-pairs x 256 cols):
  type-1 (dw=-1,0,1): K=(ch x parity of rows 2i,2i+1) -> M=(ch x parity), all
          4 quadrants of lhsT active (dh in {-1,0,+1} between the parities).
  type-2 (dw=-1,0,1): rows 2i+2 (even block) -> odd outputs (dh=+1), and rows
          2i-1 (odd block, via zs2 shift) -> even outputs (dh=-1).
Epilogue: ACT Prelu(scale*S + pb0, alpha) straight out of PSUM (or a
Relu-based decomposition, sim-friendly), then one DVE scalar_tensor_tensor:
  out = (g + pb1) + x.
"""

import os
from contextlib import ExitStack

import numpy as np

import ml_dtypes

import concourse.bass as bass
import concourse.mybir as mybir
import concourse.tile as tile
from concourse.bass_utils import run_bass_kernel_spmd
# ---------------------------------------------------------------------------
# Workaround: the in-container walrus rejects instructions carrying more than
# 2 semaphore waits ("Too many sync wait commands" in setupSyncWait), but
# Tile's sem-assignment freely attaches 3+. Post-process the serialized BIR:
# move excess waits onto NoOp instructions inserted just before the carrier
# (same engine => program order preserves the happens-before).
# ---------------------------------------------------------------------------
_MAX_WAITS = 1


def _split_sync_waits(mod: dict, max_waits: int = _MAX_WAITS) -> dict:
    for fn in mod.get("functions", []):
        for bb in fn.get("blocks", []):
            out = []
            for ins in bb.get("instructions", []):
                si = ins.get("sync_info")
                waits = (si or {}).get("on_wait") or []
                if len(waits) > max_waits:
                    extra, keep = waits[:-max_waits], waits[-max_waits:]
                    for i in range(0, len(extra), max_waits):
                        out.append({
                            "debug": ins.get("debug", 0),
                            "engine": ins["engine"],
                            "ins": [],
                            "name": f"{ins['name']}_ws{i}",
                            "opcode": "NoOp",
                            "outs": [],
                            "sync_info": {
                                "on_update": [],
                                "on_wait": extra[i:i + max_waits],
                            },
                        })
                    si["on_wait"] = keep
                out.append(ins)
            bb["instructions"] = out
    return mod


_orig_to_json_bytes = bass.Bass.to_json_bytes


def _to_json_bytes_split(self):
    import orjson

    return orjson.dumps(_split_sync_waits(orjson.loads(_orig_to_json_bytes(self))))


bass.Bass.to_json_bytes = _to_json_bytes_split

F32 = mybir.dt.float32
FP8 = mybir.dt.float8e4
NP_FP8 = ml_dtypes.float8_e4m3

# consts column indices
C_B0 = 0      # move0 bias (sign pass bias)
C_SC = 1      # scale (prelu path: activation scale)
C_PB0 = 2     # pr_bias0 (prelu path: activation bias)
C_AL = 3      # prelu alpha
C_PB1 = 4     # pr_bias1 (final add, prelu path)
C_RS = 5      # (1-a)*scale        (relu path: relu scale)
C_RB = 6      # (1-a)*pb0          (relu path: relu bias)
C_VS = 7      # a*scale            (relu path: STT1 scalar)
C_VB = 8      # a*pb0 + pb1        (relu path: final scalar)
NCOL = 9

SLOT = 272  # bytes per row-pair slot in zs tensors (16-aligned, >= 258)


def _build(B_per_core: int, H: int, W: int, C: int, G: int, use_prelu: bool):
    """Builds the per-core Bass module. Returns nc."""
    assert C == 64 and W == 256
    assert H % G == 0 and G % 4 == 0
    P = G // 2            # row-pairs per chunk
    NCH = H // G          # chunks per image
    NPAIR = H // 2        # row-pairs per image
    NSLOT = NPAIR + 2

    nc = bass.Bass()
    xd = nc.declare_dram_parameter("x", [B_per_core, C, H, W], F32, isOutput=False)
    wd = nc.declare_dram_parameter("wp", [6, 128, 128], FP8, isOutput=False)
    cd = nc.declare_dram_parameter("cv", [128, NCOL], F32, isOutput=False)
    yd = nc.declare_dram_parameter("y", [B_per_core, C, H, W], F32, isOutput=True)

    with ExitStack() as ctx:
        tc = ctx.enter_context(tile.TileContext(nc))
        cpool = ctx.enter_context(tc.tile_pool(name="const", bufs=1))
        zpool = ctx.enter_context(tc.tile_pool(name="zs", bufs=1))
        xpool = ctx.enter_context(tc.tile_pool(name="xt", bufs=4))
        gpool = ctx.enter_context(tc.tile_pool(name="gt", bufs=3))
        rpool = ctx.enter_context(tc.tile_pool(name="rt", bufs=2))
        pspool = ctx.enter_context(tc.tile_pool(name="ps", bufs=7, space="PSUM"))

        # --- resident constants ---
        wsb = cpool.tile([128, 6 * 128], FP8)
        nc.sync.dma_start(
            wsb[:].rearrange("k (t m) -> k t m", m=128),
            wd[:].rearrange("t k m -> k t m"),
        )
        cvs = cpool.tile([128, NCOL], F32)
        nc.sync.dma_start(cvs[:], cd[:])

        zs1 = zpool.tile([128, NSLOT * SLOT], FP8)
        zs2 = zpool.tile([128, NSLOT * SLOT], FP8)
        zs1v = zs1[:].rearrange("p (s c) -> p s c", c=SLOT)
        zs2v = zs2[:].rearrange("p (s c) -> p s c", c=SLOT)

        # one-time pads (stay zero across both images):
        # zs1 column pads (col -1 at byte 0, col 256 at byte 257) on every slot
        nc.gpsimd.memset(zs1v[:, :, 0:1], 0.0)
        nc.gpsimd.memset(zs1v[:, :, 257:272], 0.0)
        # zs1 even-block slot NPAIR = halo row H (all-zero)
        nc.gpsimd.memset(zs1[0:64, NPAIR * SLOT:(NPAIR + 1) * SLOT], 0.0)
        # zs2 odd-block slot 1 = halo row -1 (all-zero); slot 0 never read
        nc.gpsimd.memset(zs2[64:128, SLOT:2 * SLOT], 0.0)

        def load_sign_copy(b, k):
            """DMA x chunk k (parity layout), sign into zs1, copy into zs2."""
            r0 = k * G
            xt = xpool.tile([128, P * 256], F32, name=f"xt_{b}_{k}", tag="xt")
            xtv = xt[:].rearrange("p (s c) -> p s c", c=256)
            # even rows -> parts 0..63 ; odd rows -> parts 64..127
            nc.sync.dma_start(xtv[0:64], xd[b, :, r0:r0 + G:2, :])
            nc.sync.dma_start(xtv[64:128], xd[b, :, r0 + 1:r0 + G:2, :])
            s0 = k * P
            nc.scalar.activation(
                zs1v[:, s0:s0 + P, 1:257],
                xtv[:],
                mybir.ActivationFunctionType.Sign,
                bias=cvs[:, C_B0:C_B0 + 1],
            )
            # zs2 even block: aligned copy (last chunk: +1 slot to bring the
            # zero halo slot NPAIR across)
            ncopy = P + 1 if k == NCH - 1 else P
            nc.sync.dma_start(
                zs2[0:64, s0 * SLOT:(s0 + ncopy) * SLOT],
                zs1[0:64, s0 * SLOT:(s0 + ncopy) * SLOT],
            )
            # zs2 odd block: shifted +2 slots
            nc.sync.dma_start(
                zs2[64:128, (s0 + 2) * SLOT:(s0 + P + 2) * SLOT],
                zs1[64:128, s0 * SLOT:(s0 + P) * SLOT],
            )
            return xt

        def conv_chunk(b, k, xt):
            """6 matmuls per [128,512] PSUM tile + epilogue for chunk k."""
            r0 = k * G
            gt = gpool.tile([128, P * 256], F32, name=f"gt_{b}_{k}", tag="gt")
            for t in range(P // 2):
                i0 = k * P + 2 * t
                ps = pspool.tile([128, 512], F32, name="ps")
                nmm = 6
                mi = 0
                for ty in range(2):
                    zv = zs1v if ty == 0 else zs2v
                    base = i0 if ty == 0 else i0 + 1
                    for dw in (-1, 0, 1):
                        rhs = zv[:, base:base + 2, dw + 1:dw + 257]
                        nc.tensor.matmul(
                            ps[:],
                            wsb[:, (ty * 3 + dw + 1) * 128:(ty * 3 + dw + 2) * 128],
                            rhs,
                            start=(mi == 0),
                            stop=(mi == nmm - 1),
                        )
                        mi += 1
                gslice = gt[:, t * 512:(t + 1) * 512]
                if use_prelu:
                    nc.scalar.activation(
                        gslice,
                        ps[:],
                        mybir.ActivationFunctionType.Prelu,
                        bias=cvs[:, C_PB0:C_PB0 + 1],
                        scale=cvs[:, C_SC:C_SC + 1],
                        alpha=cvs[:, C_AL:C_AL + 1],
                    )
                else:
                    rt = rpool.tile([128, 512], F32, name="rt")
                    nc.scalar.activation(
                        rt[:],
                        ps[:],
                        mybir.ActivationFunctionType.Relu,
                        bias=cvs[:, C_RB:C_RB + 1],
                        scale=cvs[:, C_RS:C_RS + 1],
                    )
                    # g = a*scale*S + r   (r = (1-a)*relu(scale*S+pb0))
                    nc.vector.scalar_tensor_tensor(
                        gslice,
                        ps[:],
                        cvs[:, C_VS:C_VS + 1],
                        rt[:],
                        op0=mybir.AluOpType.mult,
                        op1=mybir.AluOpType.add,
                    )
            # final = (g + c) + x, in place over gt
            ccol = C_PB1 if use_prelu else C_VB
            nc.vector.scalar_tensor_tensor(
                gt[:],
                gt[:],
                cvs[:, ccol:ccol + 1],
                xt[:],
                op0=mybir.AluOpType.add,
                op1=mybir.AluOpType.add,
            )
            finv = gt[:].rearrange("p (s c) -> p s c", c=256)
            nc.sync.dma_start(yd[b, :, r0:r0 + G:2, :], finv[0:64])
            nc.sync.dma_start(yd[b, :, r0 + 1:r0 + G:2, :], finv[64:128])

        # software pipeline: run loads/sign/copies L chunks ahead of the
        # matmul+epilogue consumer so the PE never starves at chunk edges.
        LOOKAHEAD = 2
        jobs = [(b, k) for b in range(B_per_core) for k in range(NCH)]
        xts = {}
        for idx, (b, k) in enumerate(jobs):
            xts[(b, k)] = load_sign_copy(b, k)
            if idx >= LOOKAHEAD:
                bb, kk = jobs[idx - LOOKAHEAD]
                conv_chunk(bb, kk, xts.pop((bb, kk)))
        for bb, kk in jobs[-LOOKAHEAD:]:
            conv_chunk(bb, kk, xts.pop((bb, kk)))

    return nc


def _host_prep(move0_bias, conv_weight, prelu_weight, pr_bias0, pr_bias1):
    """Pack weights into the 6 lhsT matrices + per-partition constant vectors."""
    w = np.asarray(conv_weight, dtype=np.float32)          # [O, I, 3, 3]
    sw = np.sign(w).astype(np.float32)                     # {-1, 0, 1}
    scale = np.mean(np.abs(w), axis=(1, 2, 3)).astype(np.float32)  # [O]
    a = np.asarray(prelu_weight, dtype=np.float32).reshape(64)
    pb0 = np.asarray(pr_bias0, dtype=np.float32).reshape(64)
    pb1 = np.asarray(pr_bias1, dtype=np.float32).reshape(64)
    b0 = np.asarray(move0_bias, dtype=np.float32).reshape(64)

    # lhsT[k, m] with k = pi*64 + ci, m = po*64 + co ->  sw[co, ci, kh, kw]
    # type-1: dh = [[0, -1], [1, 0]][pi][po]; type-2: only (pi0,po1)=+1,(pi1,po0)=-1
    wp = np.zeros((6, 128, 128), dtype=np.float32)
    swT = np.transpose(sw, (1, 0, 2, 3))  # [ci, co, kh, kw]
    for idw, dw in enumerate((-1, 0, 1)):
        kw = dw + 1
        # type-1
        wp[idw, 0:64, 0:64] = swT[:, :, 1, kw]      # even->even  dh=0
        wp[idw, 0:64, 64:128] = swT[:, :, 0, kw]    # even->odd   dh=-1 (kh=0)
        wp[idw, 64:128, 0:64] = swT[:, :, 2, kw]    # odd->even   dh=+1 (kh=2)
        wp[idw, 64:128, 64:128] = swT[:, :, 1, kw]  # odd->odd    dh=0
        # type-2
        wp[3 + idw, 0:64, 64:128] = swT[:, :, 2, kw]   # row 2i+2 -> out 2i+1, dh=+1
        wp[3 + idw, 64:128, 0:64] = swT[:, :, 0, kw]   # row 2i-1 -> out 2i,   dh=-1
    wp8 = wp.astype(NP_FP8)

    cv = np.zeros((128, NCOL), dtype=np.float32)
    for blk in range(2):
        s = slice(blk * 64, blk * 64 + 64)
        cv[s, C_B0] = b0
        cv[s, C_SC] = scale
        cv[s, C_PB0] = pb0
        cv[s, C_AL] = a
        cv[s, C_PB1] = pb1
        cv[s, C_RS] = (1.0 - a) * scale
        cv[s, C_RB] = (1.0 - a) * pb0
        cv[s, C_VS] = a * scale
        cv[s, C_VB] = a * pb0 + pb1
    return wp8, cv


_NC_CACHE: dict = {}


def _get_nc(key, *args):
    if key not in _NC_CACHE:
        _NC_CACHE[key] = _build(*args)
    return _NC_CACHE[key]


def kernel(x, move0_bias, conv_weight, prelu_weight, pr_bias0, pr_bias1):
    x = np.asarray(x, dtype=np.float32)
    B, C, H, W = x.shape
    NCORES = 8
    assert B % NCORES == 0
    Bc = B // NCORES
    G = 32
    use_prelu = os.environ.get("BBCU_NO_PRELU", "0") != "1"

    wp8, cv = _host_prep(move0_bias, conv_weight, prelu_weight, pr_bias0, pr_bias1)

    key = (Bc, H, W, C, G, use_prelu)
    nc = _get_nc(key, Bc, H, W, C, G, use_prelu)

    in_maps = [
        {"x": x[i * Bc:(i + 1) * Bc], "wp": wp8, "cv": cv} for i in range(NCORES)
    ]
    res = run_bass_kernel_spmd(nc, in_maps, core_ids=list(range(NCORES)))
    out = np.concatenate([res.results[i]["y"] for i in range(NCORES)], axis=0)
    return out.astype(np.float32)


# revision 13
# speedup vs baseline: 1.1966x; 1.1966x over previous
"""
Binary Conv2d (BBCU-style) block on 8 Trainium2 NeuronCores.

Computation (per reference):
    z  = sign(x + move0_bias)                    # binarized activations in {-1,0,1}
    bw = scale[o] * sign(W)                      # binarized weights, per-out-channel scale
    y  = conv3x3(z, bw, pad=1)
    y  = prelu(y + pr_bias0, a) + pr_bias1 + x   # RPReLU + identity

Key exactness trick: the conv operands are exact small values (z in {-1,0,1},
sign(W) in {-1,0,1}) so we run the conv as fp8e4 matmuls with fp32 PSUM
accumulation — bit-exact integer counts (|sum| <= 576 << 2^24). The per-channel
`scale` folds into the epilogue affine constants.

Sharding: data-parallel over batch. 16 images / 8 cores = 2 images per core.

Per-core layout ("parity layout"): partitions = 64 channels x row-parity
(parts 0-63 even rows, 64-127 odd rows). SBUF tensors:
  xt   [128, P*256] f32   : chunk of G=2P rows of x (per-chunk, identity + sign input)
  zs1  [128, 130*272] fp8 : sign values, one 272B slot per row-pair index;
                            byte j in a slot = col j-1 (byte 0 / 257 = zero pad)
  zs2  [128, 130*272] fp8 : copy of zs1 with the odd block shifted +2 slots,
                            so the "cross-pair" matmuls read an aligned AP.
Conv = 6 matmuls per PSUM tile [128,512] (2 row-pairs x 256 cols):
  type-1 (dw=-1,0,1): K=(ch x parity of rows 2i,2i+1) -> M=(ch x parity), all
          4 quadrants of lhsT active (dh in {-1,0,+1} between the parities).
  type-2 (dw=-1,0,1): rows 2i+2 (even block) -> odd outputs (dh=+1), and rows
          2i-1 (odd block, via zs2 shift) -> even outputs (dh=-1).
Epilogue: ACT Prelu(scale*S + pb0, alpha) straight out of PSUM (or a
Relu-based decomposition, sim-friendly), then one DVE scalar_tensor_tensor:
  out = (g + pb1) + x.
"""

import os
from contextlib import ExitStack

import numpy as np

import ml_dtypes

import concourse.bass as bass
import concourse.mybir as mybir
import concourse.tile as tile
from concourse.bass_utils import run_bass_kernel_spmd
# ---------------------------------------------------------------------------
# Workaround: the in-container walrus rejects instructions carrying more than
# 2 semaphore waits ("Too many sync wait commands" in setupSyncWait), but
# Tile's sem-assignment freely attaches 3+. Post-process the serialized BIR:
# move excess waits onto NoOp instructions inserted just before the carrier
# (same engine => program order preserves the happens-before).
# ---------------------------------------------------------------------------
_MAX_WAITS = 1


def _split_sync_waits(mod: dict, max_waits: int = _MAX_WAITS) -> dict:
    for fn in mod.get("functions", []):
        for bb in fn.get("blocks", []):
            out = []
            for ins in bb.get("instructions", []):
                si = ins.get("sync_info")
                waits = (si or {}).get("on_wait") or []
                if len(waits) > max_waits:
                    extra, keep = waits[:-max_waits], waits[-max_waits:]
                    for i in range(0, len(extra), max_waits):
                        out.append({
                            "debug": ins.get("debug", 0),
                            "engine": ins["engine"],
                            "ins": [],
                            "name": f"{ins['name']}_ws{i}",
                            "opcode": "NoOp",
                            "outs": [],
                            "sync_info": {
                                "on_update": [],
                                "on_wait": extra[i:i + max_waits],
                            },
                        })
                    si["on_wait"] = keep
                out.append(ins)
            bb["instructions"] = out
    return mod


_orig_to_json_bytes = bass.Bass.to_json_bytes


def _to_json_bytes_split(self):
    import orjson

    return orjson.dumps(_split_sync_waits(orjson.loads(_orig_to_json_bytes(self))))


bass.Bass.to_json_bytes = _to_json_bytes_split

F32 = mybir.dt.float32
FP8 = mybir.dt.float8e4
NP_FP8 = ml_dtypes.float8_e4m3

# consts column indices
C_B0 = 0      # move0 bias (sign pass bias)
C_SC = 1      # scale (prelu path: activation scale)
C_PB0 = 2     # pr_bias0 (prelu path: activation bias)
C_AL = 3      # prelu alpha
C_PB1 = 4     # pr_bias1 (final add, prelu path)
C_RS = 5      # (1-a)*scale        (relu path: relu scale)
C_RB = 6      # (1-a)*pb0          (relu path: relu bias)
C_VS = 7      # a*scale            (relu path: STT1 scalar)
C_VB = 8      # a*pb0 + pb1        (relu path: final scalar)
NCOL = 9

SLOT = 272  # bytes per row-pair slot in zs tensors (16-aligned, >= 258)


def _build(B_per_core: int, H: int, W: int, C: int, G: int, use_prelu: bool):
    """Builds the per-core Bass module. Returns nc."""
    assert C == 64 and W == 256
    assert H % G == 0 and G % 4 == 0
    P = G // 2            # row-pairs per chunk
    NCH = H // G          # chunks per image
    NPAIR = H // 2        # row-pairs per image
    NSLOT = NPAIR + 2

    nc = bass.Bass()
    xd = nc.declare_dram_parameter("x", [B_per_core, C, H, W], F32, isOutput=False)
    wd = nc.declare_dram_parameter("wp", [6, 128, 128], FP8, isOutput=False)
    cd = nc.declare_dram_parameter("cv", [128, NCOL], F32, isOutput=False)
    yd = nc.declare_dram_parameter("y", [B_per_core, C, H, W], F32, isOutput=True)

    with ExitStack() as ctx:
        tc = ctx.enter_context(tile.TileContext(nc))
        cpool = ctx.enter_context(tc.tile_pool(name="const", bufs=1))
        zpool = ctx.enter_context(tc.tile_pool(name="zs", bufs=1))
        xpool = ctx.enter_context(tc.tile_pool(name="xt", bufs=4))
        gpool = ctx.enter_context(tc.tile_pool(name="gt", bufs=3))
        rpool = ctx.enter_context(tc.tile_pool(name="rt", bufs=2))
        pspool = ctx.enter_context(tc.tile_pool(name="ps", bufs=7, space="PSUM"))

        # --- resident constants ---
        wsb = cpool.tile([128, 6 * 128], FP8)
        nc.sync.dma_start(
            wsb[:].rearrange("k (t m) -> k t m", m=128),
            wd[:].rearrange("t k m -> k t m"),
        )
        cvs = cpool.tile([128, NCOL], F32)
        nc.sync.dma_start(cvs[:], cd[:])

        zs1 = zpool.tile([128, NSLOT * SLOT], FP8)
        zs2 = zpool.tile([128, NSLOT * SLOT], FP8)
        zs1v = zs1[:].rearrange("p (s c) -> p s c", c=SLOT)
        zs2v = zs2[:].rearrange("p (s c) -> p s c", c=SLOT)

        # one-time pads (stay zero across both images):
        # zs1 column pads (col -1 at byte 0, col 256 at byte 257) on every slot
        nc.gpsimd.memset(zs1v[:, :, 0:1], 0.0)
        nc.gpsimd.memset(zs1v[:, :, 257:272], 0.0)
        # zs1 even-block slot NPAIR = halo row H (all-zero)
        nc.gpsimd.memset(zs1[0:64, NPAIR * SLOT:(NPAIR + 1) * SLOT], 0.0)
        # zs2 odd-block slot 1 = halo row -1 (all-zero); slot 0 never read
        nc.gpsimd.memset(zs2[64:128, SLOT:2 * SLOT], 0.0)

        def load_sign_copy(b, k):
            """DMA x chunk k (parity layout), sign into zs1, copy into zs2."""
            r0 = k * G
            xt = xpool.tile([128, P * 256], F32, name=f"xt_{b}_{k}", tag="xt")
            xtv = xt[:].rearrange("p (s c) -> p s c", c=256)
            # even rows -> parts 0..63 ; odd rows -> parts 64..127
            nc.sync.dma_start(xtv[0:64], xd[b, :, r0:r0 + G:2, :])
            nc.sync.dma_start(xtv[64:128], xd[b, :, r0 + 1:r0 + G:2, :])
            s0 = k * P
            nc.scalar.activation(
                zs1v[:, s0:s0 + P, 1:257],
                xtv[:],
                mybir.ActivationFunctionType.Sign,
                bias=cvs[:, C_B0:C_B0 + 1],
            )
            # zs2 even block: aligned copy (last chunk: +1 slot to bring the
            # zero halo slot NPAIR across). On GpSimd (SWDGE) so the wait on
            # the sign op never head-of-line-blocks the SP load stream.
            ncopy = P + 1 if k == NCH - 1 else P
            nc.gpsimd.dma_start(
                zs2[0:64, s0 * SLOT:(s0 + ncopy) * SLOT],
                zs1[0:64, s0 * SLOT:(s0 + ncopy) * SLOT],
            )
            # zs2 odd block: shifted +2 slots
            nc.gpsimd.dma_start(
                zs2[64:128, (s0 + 2) * SLOT:(s0 + P + 2) * SLOT],
                zs1[64:128, s0 * SLOT:(s0 + P) * SLOT],
            )
            return xt

        def conv_chunk(b, k, xt):
            """6 matmuls per [128,512] PSUM tile + epilogue for chunk k."""
            r0 = k * G
            gt = gpool.tile([128, P * 256], F32, name=f"gt_{b}_{k}", tag="gt")
            for t in range(P // 2):
                i0 = k * P + 2 * t
                ps = pspool.tile([128, 512], F32, name="ps")
                nmm = 6
                mi = 0
                for ty in range(2):
                    zv = zs1v if ty == 0 else zs2v
                    base = i0 if ty == 0 else i0 + 1
                    for dw in (-1, 0, 1):
                        rhs = zv[:, base:base + 2, dw + 1:dw + 257]
                        nc.tensor.matmul(
                            ps[:],
                            wsb[:, (ty * 3 + dw + 1) * 128:(ty * 3 + dw + 2) * 128],
                            rhs,
                            start=(mi == 0),
                            stop=(mi == nmm - 1),
                        )
                        mi += 1
                gslice = gt[:, t * 512:(t + 1) * 512]
                if use_prelu:
                    nc.scalar.activation(
                        gslice,
                        ps[:],
                        mybir.ActivationFunctionType.Prelu,
                        bias=cvs[:, C_PB0:C_PB0 + 1],
                        scale=cvs[:, C_SC:C_SC + 1],
                        alpha=cvs[:, C_AL:C_AL + 1],
                    )
                else:
                    rt = rpool.tile([128, 512], F32, name="rt")
                    nc.scalar.activation(
                        rt[:],
                        ps[:],
                        mybir.ActivationFunctionType.Relu,
                        bias=cvs[:, C_RB:C_RB + 1],
                        scale=cvs[:, C_RS:C_RS + 1],
                    )
                    # g = a*scale*S + r   (r = (1-a)*relu(scale*S+pb0))
                    nc.vector.scalar_tensor_tensor(
                        gslice,
                        ps[:],
                        cvs[:, C_VS:C_VS + 1],
                        rt[:],
                        op0=mybir.AluOpType.mult,
                        op1=mybir.AluOpType.add,
                    )
            # final = (g + c) + x, in place over gt
            ccol = C_PB1 if use_prelu else C_VB
            nc.vector.scalar_tensor_tensor(
                gt[:],
                gt[:],
                cvs[:, ccol:ccol + 1],
                xt[:],
                op0=mybir.AluOpType.add,
                op1=mybir.AluOpType.add,
            )
            finv = gt[:].rearrange("p (s c) -> p s c", c=256)
            nc.gpsimd.dma_start(yd[b, :, r0:r0 + G:2, :], finv[0:64])
            nc.gpsimd.dma_start(yd[b, :, r0 + 1:r0 + G:2, :], finv[64:128])

        # software pipeline: run loads/sign/copies L chunks ahead of the
        # matmul+epilogue consumer so the PE never starves at chunk edges.
        LOOKAHEAD = 2
        jobs = [(b, k) for b in range(B_per_core) for k in range(NCH)]
        xts = {}
        for idx, (b, k) in enumerate(jobs):
            xts[(b, k)] = load_sign_copy(b, k)
            if idx >= LOOKAHEAD:
                bb, kk = jobs[idx - LOOKAHEAD]
                conv_chunk(bb, kk, xts.pop((bb, kk)))
        for bb, kk in jobs[-LOOKAHEAD:]:
            conv_chunk(bb, kk, xts.pop((bb, kk)))

    return nc


def _host_prep(move0_bias, conv_weight, prelu_weight, pr_bias0, pr_bias1):
    """Pack weights into the 6 lhsT matrices + per-partition constant vectors."""
    w = np.asarray(conv_weight, dtype=np.float32)          # [O, I, 3, 3]
    sw = np.sign(w).astype(np.float32)                     # {-1, 0, 1}
    scale = np.mean(np.abs(w), axis=(1, 2, 3)).astype(np.float32)  # [O]
    a = np.asarray(prelu_weight, dtype=np.float32).reshape(64)
    pb0 = np.asarray(pr_bias0, dtype=np.float32).reshape(64)
    pb1 = np.asarray(pr_bias1, dtype=np.float32).reshape(64)
    b0 = np.asarray(move0_bias, dtype=np.float32).reshape(64)

    # lhsT[k, m] with k = pi*64 + ci, m = po*64 + co ->  sw[co, ci, kh, kw]
    # type-1: dh = [[0, -1], [1, 0]][pi][po]; type-2: only (pi0,po1)=+1,(pi1,po0)=-1
    wp = np.zeros((6, 128, 128), dtype=np.float32)
    swT = np.transpose(sw, (1, 0, 2, 3))  # [ci, co, kh, kw]
    for idw, dw in enumerate((-1, 0, 1)):
        kw = dw + 1
        # type-1
        wp[idw, 0:64, 0:64] = swT[:, :, 1, kw]      # even->even  dh=0
        wp[idw, 0:64, 64:128] = swT[:, :, 0, kw]    # even->odd   dh=-1 (kh=0)
        wp[idw, 64:128, 0:64] = swT[:, :, 2, kw]    # odd->even   dh=+1 (kh=2)
        wp[idw, 64:128, 64:128] = swT[:, :, 1, kw]  # odd->odd    dh=0
        # type-2
        wp[3 + idw, 0:64, 64:128] = swT[:, :, 2, kw]   # row 2i+2 -> out 2i+1, dh=+1
        wp[3 + idw, 64:128, 0:64] = swT[:, :, 0, kw]   # row 2i-1 -> out 2i,   dh=-1
    wp8 = wp.astype(NP_FP8)

    cv = np.zeros((128, NCOL), dtype=np.float32)
    for blk in range(2):
        s = slice(blk * 64, blk * 64 + 64)
        cv[s, C_B0] = b0
        cv[s, C_SC] = scale
        cv[s, C_PB0] = pb0
        cv[s, C_AL] = a
        cv[s, C_PB1] = pb1
        cv[s, C_RS] = (1.0 - a) * scale
        cv[s, C_RB] = (1.0 - a) * pb0
        cv[s, C_VS] = a * scale
        cv[s, C_VB] = a * pb0 + pb1
    return wp8, cv


_NC_CACHE: dict = {}


def _get_nc(key, *args):
    if key not in _NC_CACHE:
        _NC_CACHE[key] = _build(*args)
    return _NC_CACHE[key]


def kernel(x, move0_bias, conv_weight, prelu_weight, pr_bias0, pr_bias1):
    x = np.asarray(x, dtype=np.float32)
    B, C, H, W = x.shape
    NCORES = 8
    assert B % NCORES == 0
    Bc = B // NCORES
    G = 32
    use_prelu = os.environ.get("BBCU_NO_PRELU", "0") != "1"

    wp8, cv = _host_prep(move0_bias, conv_weight, prelu_weight, pr_bias0, pr_bias1)

    key = (Bc, H, W, C, G, use_prelu)
    nc = _get_nc(key, Bc, H, W, C, G, use_prelu)

    in_maps = [
        {"x": x[i * Bc:(i + 1) * Bc], "wp": wp8, "cv": cv} for i in range(NCORES)
    ]
    res = run_bass_kernel_spmd(nc, in_maps, core_ids=list(range(NCORES)))
    out = np.concatenate([res.results[i]["y"] for i in range(NCORES)], axis=0)
    return out.astype(np.float32)


# revision 15
# speedup vs baseline: 1.2333x; 1.0307x over previous
"""
Binary Conv2d (BBCU-style) block on 8 Trainium2 NeuronCores.

Computation (per reference):
    z  = sign(x + move0_bias)                    # binarized activations in {-1,0,1}
    bw = scale[o] * sign(W)                      # binarized weights, per-out-channel scale
    y  = conv3x3(z, bw, pad=1)
    y  = prelu(y + pr_bias0, a) + pr_bias1 + x   # RPReLU + identity

Key exactness trick: the conv operands are exact small values (z in {-1,0,1},
sign(W) in {-1,0,1}) so we run the conv as fp8e4 matmuls with fp32 PSUM
accumulation — bit-exact integer counts (|sum| <= 576 << 2^24). The per-channel
`scale` folds into the epilogue affine constants.

Sharding: data-parallel over batch. 16 images / 8 cores = 2 images per core.

Per-core layout ("parity layout"): partitions = 64 channels x row-parity
(parts 0-63 even rows, 64-127 odd rows). SBUF tensors:
  xt   [128, P*256] f32   : chunk of G=2P rows of x (per-chunk, identity + sign input)
  zs1  [128, 130*272] fp8 : sign values, one 272B slot per row-pair index;
                            byte j in a slot = col j-1 (byte 0 / 257 = zero pad)
  zs2  [128, 130*272] fp8 : copy of zs1 with the odd block shifted +2 slots,
                            so the "cross-pair" matmuls read an aligned AP.
Conv = 6 matmuls per PSUM tile [128,512] (2 row-pairs x 256 cols):
  type-1 (dw=-1,0,1): K=(ch x parity of rows 2i,2i+1) -> M=(ch x parity), all
          4 quadrants of lhsT active (dh in {-1,0,+1} between the parities).
  type-2 (dw=-1,0,1): rows 2i+2 (even block) -> odd outputs (dh=+1), and rows
          2i-1 (odd block, via zs2 shift) -> even outputs (dh=-1).
Epilogue: ACT Prelu(scale*S + pb0, alpha) straight out of PSUM (or a
Relu-based decomposition, sim-friendly), then one DVE scalar_tensor_tensor:
  out = (g + pb1) + x.
"""

import os
from contextlib import ExitStack

import numpy as np

import ml_dtypes

import concourse.bass as bass
import concourse.mybir as mybir
import concourse.tile as tile
from concourse.bass_utils import run_bass_kernel_spmd
# ---------------------------------------------------------------------------
# Workaround: the in-container walrus rejects instructions carrying more than
# 2 semaphore waits ("Too many sync wait commands" in setupSyncWait), but
# Tile's sem-assignment freely attaches 3+. Post-process the serialized BIR:
# move excess waits onto NoOp instructions inserted just before the carrier
# (same engine => program order preserves the happens-before).
# ---------------------------------------------------------------------------
_MAX_WAITS = 1


def _split_sync_waits(mod: dict, max_waits: int = _MAX_WAITS) -> dict:
    for fn in mod.get("functions", []):
        for bb in fn.get("blocks", []):
            out = []
            for ins in bb.get("instructions", []):
                si = ins.get("sync_info")
                waits = (si or {}).get("on_wait") or []
                if len(waits) > max_waits:
                    extra, keep = waits[:-max_waits], waits[-max_waits:]
                    for i in range(0, len(extra), max_waits):
                        out.append({
                            "debug": ins.get("debug", 0),
                            "engine": ins["engine"],
                            "ins": [],
                            "name": f"{ins['name']}_ws{i}",
                            "opcode": "NoOp",
                            "outs": [],
                            "sync_info": {
                                "on_update": [],
                                "on_wait": extra[i:i + max_waits],
                            },
                        })
                    si["on_wait"] = keep
                out.append(ins)
            bb["instructions"] = out
    return mod


_orig_to_json_bytes = bass.Bass.to_json_bytes


def _to_json_bytes_split(self):
    import orjson

    return orjson.dumps(_split_sync_waits(orjson.loads(_orig_to_json_bytes(self))))


bass.Bass.to_json_bytes = _to_json_bytes_split

F32 = mybir.dt.float32
FP8 = mybir.dt.float8e4
NP_FP8 = ml_dtypes.float8_e4m3

# consts column indices
C_B0 = 0      # move0 bias (sign pass bias)
C_SC = 1      # scale (prelu path: activation scale)
C_PB0 = 2     # pr_bias0 (prelu path: activation bias)
C_AL = 3      # prelu alpha
C_PB1 = 4     # pr_bias1 (final add, prelu path)
C_RS = 5      # (1-a)*scale        (relu path: relu scale)
C_RB = 6      # (1-a)*pb0          (relu path: relu bias)
C_VS = 7      # a*scale            (relu path: STT1 scalar)
C_VB = 8      # a*pb0 + pb1        (relu path: final scalar)
NCOL = 9

SLOT = 272  # bytes per row-pair slot in zs tensors (16-aligned, >= 258)


def _build(B_per_core: int, H: int, W: int, C: int, G: int, use_prelu: bool):
    """Builds the per-core Bass module. Returns nc."""
    assert C == 64 and W == 256
    assert H % G == 0 and G % 4 == 0
    P = G // 2            # row-pairs per chunk
    NCH = H // G          # chunks per image
    NPAIR = H // 2        # row-pairs per image
    NSLOT = NPAIR + 2

    nc = bass.Bass()
    xd = nc.declare_dram_parameter("x", [B_per_core, C, H, W], F32, isOutput=False)
    wd = nc.declare_dram_parameter("wp", [6, 128, 128], FP8, isOutput=False)
    cd = nc.declare_dram_parameter("cv", [128, NCOL], F32, isOutput=False)
    yd = nc.declare_dram_parameter("y", [B_per_core, C, H, W], F32, isOutput=True)

    with ExitStack() as ctx:
        tc = ctx.enter_context(tile.TileContext(nc))
        cpool = ctx.enter_context(tc.tile_pool(name="const", bufs=1))
        zpool = ctx.enter_context(tc.tile_pool(name="zs", bufs=1))
        xpool = ctx.enter_context(tc.tile_pool(name="xt", bufs=4))
        gpool = ctx.enter_context(tc.tile_pool(name="gt", bufs=3))
        rpool = ctx.enter_context(tc.tile_pool(name="rt", bufs=2))
        pspool = ctx.enter_context(tc.tile_pool(name="ps", bufs=7, space="PSUM"))

        # --- resident constants ---
        wsb = cpool.tile([128, 6 * 128], FP8)
        nc.sync.dma_start(
            wsb[:].rearrange("k (t m) -> k t m", m=128),
            wd[:].rearrange("t k m -> k t m"),
        )
        cvs = cpool.tile([128, NCOL], F32)
        nc.sync.dma_start(cvs[:], cd[:])

        zs1 = zpool.tile([128, NSLOT * SLOT], FP8)
        zs2 = zpool.tile([128, NSLOT * SLOT], FP8)
        zs1v = zs1[:].rearrange("p (s c) -> p s c", c=SLOT)
        zs2v = zs2[:].rearrange("p (s c) -> p s c", c=SLOT)

        # one-time pads (stay zero across both images):
        # zs1 column pads (col -1 at byte 0, col 256 at byte 257) on every slot
        nc.gpsimd.memset(zs1v[:, :, 0:1], 0.0)
        nc.gpsimd.memset(zs1v[:, :, 257:272], 0.0)
        # zs1 even-block slot NPAIR = halo row H (all-zero)
        nc.gpsimd.memset(zs1[0:64, NPAIR * SLOT:(NPAIR + 1) * SLOT], 0.0)
        # zs2 odd-block slot 1 = halo row -1 (all-zero); slot 0 never read
        nc.gpsimd.memset(zs2[64:128, SLOT:2 * SLOT], 0.0)

        def load_sign_copy(b, k):
            """DMA x chunk k (parity layout), sign into zs1, copy into zs2."""
            r0 = k * G
            xt = xpool.tile([128, P * 256], F32, name=f"xt_{b}_{k}", tag="xt")
            xtv = xt[:].rearrange("p (s c) -> p s c", c=256)
            # even rows -> parts 0..63 ; odd rows -> parts 64..127
            nc.sync.dma_start(xtv[0:64], xd[b, :, r0:r0 + G:2, :])
            nc.sync.dma_start(xtv[64:128], xd[b, :, r0 + 1:r0 + G:2, :])
            s0 = k * P
            nc.scalar.activation(
                zs1v[:, s0:s0 + P, 1:257],
                xtv[:],
                mybir.ActivationFunctionType.Sign,
                bias=cvs[:, C_B0:C_B0 + 1],
            )
            return xt

        def do_copies(b, k):
            # zs1 -> zs2 copies for chunk k. Emitted one pipeline step after
            # the sign op so the RAW wait on sign is already satisfied and
            # never head-of-line-blocks the SP DMA stream.
            s0 = k * P
            # even block: aligned copy (last chunk: +1 slot to bring the
            # zero halo slot NPAIR across)
            ncopy = P + 1 if k == NCH - 1 else P
            nc.sync.dma_start(
                zs2[0:64, s0 * SLOT:(s0 + ncopy) * SLOT],
                zs1[0:64, s0 * SLOT:(s0 + ncopy) * SLOT],
            )
            # odd block: shifted +2 slots
            nc.sync.dma_start(
                zs2[64:128, (s0 + 2) * SLOT:(s0 + P + 2) * SLOT],
                zs1[64:128, s0 * SLOT:(s0 + P) * SLOT],
            )

        def conv_chunk(b, k, xt):
            """6 matmuls per [128,512] PSUM tile + epilogue for chunk k."""
            r0 = k * G
            gt = gpool.tile([128, P * 256], F32, name=f"gt_{b}_{k}", tag="gt")
            for t in range(P // 2):
                i0 = k * P + 2 * t
                ps = pspool.tile([128, 512], F32, name="ps")
                nmm = 6
                mi = 0
                for ty in range(2):
                    zv = zs1v if ty == 0 else zs2v
                    base = i0 if ty == 0 else i0 + 1
                    for dw in (-1, 0, 1):
                        rhs = zv[:, base:base + 2, dw + 1:dw + 257]
                        nc.tensor.matmul(
                            ps[:],
                            wsb[:, (ty * 3 + dw + 1) * 128:(ty * 3 + dw + 2) * 128],
                            rhs,
                            start=(mi == 0),
                            stop=(mi == nmm - 1),
                        )
                        mi += 1
                gslice = gt[:, t * 512:(t + 1) * 512]
                if use_prelu:
                    nc.scalar.activation(
                        gslice,
                        ps[:],
                        mybir.ActivationFunctionType.Prelu,
                        bias=cvs[:, C_PB0:C_PB0 + 1],
                        scale=cvs[:, C_SC:C_SC + 1],
                        alpha=cvs[:, C_AL:C_AL + 1],
                    )
                else:
                    rt = rpool.tile([128, 512], F32, name="rt")
                    nc.scalar.activation(
                        rt[:],
                        ps[:],
                        mybir.ActivationFunctionType.Relu,
                        bias=cvs[:, C_RB:C_RB + 1],
                        scale=cvs[:, C_RS:C_RS + 1],
                    )
                    # g = a*scale*S + r   (r = (1-a)*relu(scale*S+pb0))
                    nc.vector.scalar_tensor_tensor(
                        gslice,
                        ps[:],
                        cvs[:, C_VS:C_VS + 1],
                        rt[:],
                        op0=mybir.AluOpType.mult,
                        op1=mybir.AluOpType.add,
                    )
            # final = (g + c) + x, in place over gt
            ccol = C_PB1 if use_prelu else C_VB
            nc.vector.scalar_tensor_tensor(
                gt[:],
                gt[:],
                cvs[:, ccol:ccol + 1],
                xt[:],
                op0=mybir.AluOpType.add,
                op1=mybir.AluOpType.add,
            )
            finv = gt[:].rearrange("p (s c) -> p s c", c=256)
            nc.gpsimd.dma_start(yd[b, :, r0:r0 + G:2, :], finv[0:64])
            nc.gpsimd.dma_start(yd[b, :, r0 + 1:r0 + G:2, :], finv[64:128])

        # software pipeline: loads/sign run 2 chunks ahead, copies 1 ahead,
        # matmuls+epilogue consume — the PE never starves at chunk edges.
        jobs = [(b, k) for b in range(B_per_core) for k in range(NCH)]
        xts = {}
        for idx, (b, k) in enumerate(jobs):
            xts[(b, k)] = load_sign_copy(b, k)
            if idx >= 1:
                do_copies(*jobs[idx - 1])
            if idx >= 2:
                bb, kk = jobs[idx - 2]
                conv_chunk(bb, kk, xts.pop((bb, kk)))
        do_copies(*jobs[-1])
        for bb, kk in jobs[-2:]:
            conv_chunk(bb, kk, xts.pop((bb, kk)))

    return nc


def _host_prep(move0_bias, conv_weight, prelu_weight, pr_bias0, pr_bias1):
    """Pack weights into the 6 lhsT matrices + per-partition constant vectors."""
    w = np.asarray(conv_weight, dtype=np.float32)          # [O, I, 3, 3]
    sw = np.sign(w).astype(np.float32)                     # {-1, 0, 1}
    scale = np.mean(np.abs(w), axis=(1, 2, 3)).astype(np.float32)  # [O]
    a = np.asarray(prelu_weight, dtype=np.float32).reshape(64)
    pb0 = np.asarray(pr_bias0, dtype=np.float32).reshape(64)
    pb1 = np.asarray(pr_bias1, dtype=np.float32).reshape(64)
    b0 = np.asarray(move0_bias, dtype=np.float32).reshape(64)

    # lhsT[k, m] with k = pi*64 + ci, m = po*64 + co ->  sw[co, ci, kh, kw]
    # type-1: dh = [[0, -1], [1, 0]][pi][po]; type-2: only (pi0,po1)=+1,(pi1,po0)=-1
    wp = np.zeros((6, 128, 128), dtype=np.float32)
    swT = np.transpose(sw, (1, 0, 2, 3))  # [ci, co, kh, kw]
    for idw, dw in enumerate((-1, 0, 1)):
        kw = dw + 1
        # type-1
        wp[idw, 0:64, 0:64] = swT[:, :, 1, kw]      # even->even  dh=0
        wp[idw, 0:64, 64:128] = swT[:, :, 0, kw]    # even->odd   dh=-1 (kh=0)
        wp[idw, 64:128, 0:64] = swT[:, :, 2, kw]    # odd->even   dh=+1 (kh=2)
        wp[idw, 64:128, 64:128] = swT[:, :, 1, kw]  # odd->odd    dh=0
        # type-2
        wp[3 + idw, 0:64, 64:128] = swT[:, :, 2, kw]   # row 2i+2 -> out 2i+1, dh=+1
        wp[3 + idw, 64:128, 0:64] = swT[:, :, 0, kw]   # row 2i-1 -> out 2i,   dh=-1
    wp8 = wp.astype(NP_FP8)

    cv = np.zeros((128, NCOL), dtype=np.float32)
    for blk in range(2):
        s = slice(blk * 64, blk * 64 + 64)
        cv[s, C_B0] = b0
        cv[s, C_SC] = scale
        cv[s, C_PB0] = pb0
        cv[s, C_AL] = a
        cv[s, C_PB1] = pb1
        cv[s, C_RS] = (1.0 - a) * scale
        cv[s, C_RB] = (1.0 - a) * pb0
        cv[s, C_VS] = a * scale
        cv[s, C_VB] = a * pb0 + pb1
    return wp8, cv


_NC_CACHE: dict = {}


def _get_nc(key, *args):
    if key not in _NC_CACHE:
        _NC_CACHE[key] = _build(*args)
    return _NC_CACHE[key]


def kernel(x, move0_bias, conv_weight, prelu_weight, pr_bias0, pr_bias1):
    x = np.asarray(x, dtype=np.float32)
    B, C, H, W = x.shape
    NCORES = 8
    assert B % NCORES == 0
    Bc = B // NCORES
    G = 32
    use_prelu = os.environ.get("BBCU_NO_PRELU", "0") != "1"

    wp8, cv = _host_prep(move0_bias, conv_weight, prelu_weight, pr_bias0, pr_bias1)

    key = (Bc, H, W, C, G, use_prelu)
    nc = _get_nc(key, Bc, H, W, C, G, use_prelu)

    in_maps = [
        {"x": x[i * Bc:(i + 1) * Bc], "wp": wp8, "cv": cv} for i in range(NCORES)
    ]
    res = run_bass_kernel_spmd(nc, in_maps, core_ids=list(range(NCORES)))
    out = np.concatenate([res.results[i]["y"] for i in range(NCORES)], axis=0)
    return out.astype(np.float32)


# revision 23
# speedup vs baseline: 1.2909x; 1.0467x over previous
"""
Binary Conv2d (BBCU-style) block on 8 Trainium2 NeuronCores.

Computation (per reference):
    z  = sign(x + move0_bias)                    # binarized activations in {-1,0,1}
    bw = scale[o] * sign(W)                      # binarized weights, per-out-channel scale
    y  = conv3x3(z, bw, pad=1)
    y  = prelu(y + pr_bias0, a) + pr_bias1 + x   # RPReLU + identity

Key exactness trick: the conv operands are exact small values (z in {-1,0,1},
sign(W) in {-1,0,1}) so we run the conv as fp8e4 matmuls with fp32 PSUM
accumulation — bit-exact integer counts (|sum| <= 576 << 2^24). The per-channel
`scale` folds into the epilogue affine constants.

Sharding: data-parallel over batch. 16 images / 8 cores = 2 images per core.

Per-core layout ("parity layout"): partitions = 64 channels x row-parity
(parts 0-63 even rows, 64-127 odd rows). SBUF tensors:
  xt   [128, P*256] f32   : chunk of G=2P rows of x (per-chunk, identity + sign input)
  zs1  [128, 130*272] fp8 : sign values, one 272B slot per row-pair index;
                            byte j in a slot = col j-1 (byte 0 / 257 = zero pad)
  zs2  [128, 130*272] fp8 : copy of zs1 with the odd block shifted +2 slots,
                            so the "cross-pair" matmuls read an aligned AP.
Conv = 6 matmuls per PSUM tile [128,512] (2 row-pairs x 256 cols):
  type-1 (dw=-1,0,1): K=(ch x parity of rows 2i,2i+1) -> M=(ch x parity), all
          4 quadrants of lhsT active (dh in {-1,0,+1} between the parities).
  type-2 (dw=-1,0,1): rows 2i+2 (even block) -> odd outputs (dh=+1), and rows
          2i-1 (odd block, via zs2 shift) -> even outputs (dh=-1).
Epilogue: ACT Prelu(scale*S + pb0, alpha) straight out of PSUM (or a
Relu-based decomposition, sim-friendly), then one DVE scalar_tensor_tensor:
  out = (g + pb1) + x.
"""

import os
from contextlib import ExitStack

import numpy as np

import ml_dtypes

import concourse.bass as bass
import concourse.mybir as mybir
import concourse.tile as tile
from concourse.bass_utils import run_bass_kernel_spmd
# ---------------------------------------------------------------------------
# Workaround: the in-container walrus rejects instructions carrying more than
# 2 semaphore waits ("Too many sync wait commands" in setupSyncWait), but
# Tile's sem-assignment freely attaches 3+. Post-process the serialized BIR:
# move excess waits onto NoOp instructions inserted just before the carrier
# (same engine => program order preserves the happens-before).
# ---------------------------------------------------------------------------
_MAX_WAITS = 1


def _split_sync_waits(mod: dict, max_waits: int = _MAX_WAITS) -> dict:
    for fn in mod.get("functions", []):
        for bb in fn.get("blocks", []):
            out = []
            for ins in bb.get("instructions", []):
                si = ins.get("sync_info")
                waits = (si or {}).get("on_wait") or []
                if len(waits) > max_waits:
                    extra, keep = waits[:-max_waits], waits[-max_waits:]
                    for i in range(0, len(extra), max_waits):
                        out.append({
                            "debug": ins.get("debug", 0),
                            "engine": ins["engine"],
                            "ins": [],
                            "name": f"{ins['name']}_ws{i}",
                            "opcode": "NoOp",
                            "outs": [],
                            "sync_info": {
                                "on_update": [],
                                "on_wait": extra[i:i + max_waits],
                            },
                        })
                    si["on_wait"] = keep
                out.append(ins)
            bb["instructions"] = out
    return mod


_orig_to_json_bytes = bass.Bass.to_json_bytes


def _to_json_bytes_split(self):
    import orjson

    return orjson.dumps(_split_sync_waits(orjson.loads(_orig_to_json_bytes(self))))


bass.Bass.to_json_bytes = _to_json_bytes_split

F32 = mybir.dt.float32
FP8 = mybir.dt.float8e4
NP_FP8 = ml_dtypes.float8_e4m3

# consts column indices
C_B0 = 0      # move0 bias (sign pass bias)
C_SC = 1      # scale (prelu path: activation scale)
C_PB0 = 2     # pr_bias0 (prelu path: activation bias)
C_AL = 3      # prelu alpha
C_PB1 = 4     # pr_bias1 (final add, prelu path)
C_RS = 5      # (1-a)*scale        (relu path: relu scale)
C_RB = 6      # (1-a)*pb0          (relu path: relu bias)
C_VS = 7      # a*scale            (relu path: STT1 scalar)
C_VB = 8      # a*pb0 + pb1        (relu path: final scalar)
NCOL = 9

SLOT = 272  # bytes per row-pair slot in zs tensors (16-aligned, >= 258)


def _build(B_per_core: int, H: int, W: int, C: int, G: int, use_prelu: bool):
    """Builds the per-core Bass module. Returns nc."""
    assert C == 64 and W == 256
    assert H % G == 0 and G % 4 == 0
    P = G // 2            # row-pairs per chunk
    NCH = H // G          # chunks per image
    NPAIR = H // 2        # row-pairs per image
    NSLOT = NPAIR + 2

    nc = bass.Bass()
    xd = nc.declare_dram_parameter("x", [B_per_core, C, H, W], F32, isOutput=False)
    wd = nc.declare_dram_parameter("wp", [6, 128, 128], FP8, isOutput=False)
    cd = nc.declare_dram_parameter("cv", [128, NCOL], F32, isOutput=False)
    yd = nc.declare_dram_parameter("y", [B_per_core, C, H, W], F32, isOutput=True)

    with ExitStack() as ctx:
        tc = ctx.enter_context(tile.TileContext(nc))
        cpool = ctx.enter_context(tc.tile_pool(name="const", bufs=1))
        zpool = ctx.enter_context(tc.tile_pool(name="zs", bufs=1))
        xpool = ctx.enter_context(tc.tile_pool(name="xt", bufs=5))
        gpool = ctx.enter_context(tc.tile_pool(name="gt", bufs=3))
        rpool = ctx.enter_context(tc.tile_pool(name="rt", bufs=2))
        pspool = ctx.enter_context(tc.tile_pool(name="ps", bufs=7, space="PSUM"))

        # --- resident constants ---
        wsb = cpool.tile([128, 6 * 128], FP8)
        nc.sync.dma_start(
            wsb[:].rearrange("k (t m) -> k t m", m=128),
            wd[:].rearrange("t k m -> k t m"),
        )
        cvs = cpool.tile([128, NCOL], F32)
        nc.sync.dma_start(cvs[:], cd[:])

        # zs1 slot j holds rows (2(j-1), 2(j-1)+1) on the (even, odd) blocks;
        # slot 0 and slot NPAIR+1 are zero halo pads.
        zs1 = zpool.tile([128, NSLOT * SLOT], FP8)
        zs1v = zs1[:].rearrange("p (s c) -> p s c", c=SLOT)

        # one-time pads (stay zero across both images):
        # column pads (col -1 at byte 0, col 256 at byte 257) on every slot
        nc.gpsimd.memset(zs1v[:, :, 0:1], 0.0)
        nc.gpsimd.memset(zs1v[:, :, 257:272], 0.0)
        # halo row slots (rows below 0 / above H-1)
        nc.gpsimd.memset(zs1[:, 0:SLOT], 0.0)
        nc.gpsimd.memset(zs1[:, (NPAIR + 1) * SLOT:(NPAIR + 2) * SLOT], 0.0)

        def load_sign_copy(b, k):
            """DMA x chunk k (parity layout), sign into zs1, copy into zs2."""
            r0 = k * G
            xt = xpool.tile([128, P * 256], F32, name=f"xt_{b}_{k}", tag="xt")
            xtv = xt[:].rearrange("p (s c) -> p s c", c=256)
            # even rows -> parts 0..63 ; odd rows -> parts 64..127
            nc.sync.dma_start(xtv[0:64], xd[b, :, r0:r0 + G:2, :])
            nc.sync.dma_start(xtv[64:128], xd[b, :, r0 + 1:r0 + G:2, :])
            s0 = k * P + 1
            nc.scalar.activation(
                zs1v[:, s0:s0 + P, 1:257],
                xtv[:],
                mybir.ActivationFunctionType.Sign,
                bias=cvs[:, C_B0:C_B0 + 1],
            )
            return xt

        def conv_chunk(b, k, xt):
            """6 matmuls per [128,512] PSUM tile + epilogue for chunk k."""
            r0 = k * G
            gt = gpool.tile([128, P * 256], F32, name=f"gt_{b}_{k}", tag="gt")
            for t in range(P // 2):
                i0 = k * P + 2 * t
                ps = pspool.tile([128, 512], F32, name="ps")
                # 3 full-array type-1 matmuls (rows 2i..2i+3 -> same pair)
                for mi, dw in enumerate((-1, 0, 1)):
                    rhs = zs1v[:, i0 + 1:i0 + 3, dw + 1:dw + 257]
                    nc.tensor.matmul(
                        ps[:],
                        wsb[:, (dw + 1) * 128:(dw + 2) * 128],
                        rhs,
                        start=(mi == 0),
                        stop=(mi == 2),
                    )
                # cross-pair contributions as pairs of concurrent quadrant
                # matmuls (disjoint 64x64 array tiles, own rhs offsets):
                #   a: even rows 2i+2/2i+4 -> odd outputs   (dh=+1)
                #   b: odd rows 2i-1/2i+1  -> even outputs  (dh=-1)
                for mi, dw in enumerate((-1, 0, 1)):
                    wcol = (3 + dw + 1) * 128
                    # skip_group_check: CoreSim's PSUM-group table mis-addresses
                    # base_partition != 0 outputs; HW accumulation is per-element
                    # has_written and is correct. start/stop live on the type-1
                    # full-array group above.
                    nc.tensor.matmul(
                        ps[64:128, :],
                        wsb[0:64, wcol + 64:wcol + 128],
                        zs1v[0:64, i0 + 2:i0 + 4, dw + 1:dw + 257],
                        start=False,
                        stop=False,
                        skip_group_check=True,
                        tile_position=(0, 64),
                    )
                    nc.tensor.matmul(
                        ps[0:64, :],
                        wsb[64:128, wcol:wcol + 64],
                        zs1v[64:128, i0:i0 + 2, dw + 1:dw + 257],
                        start=False,
                        stop=False,
                        skip_group_check=True,
                        tile_position=(64, 0),
                    )
                gslice = gt[:, t * 512:(t + 1) * 512]
                if use_prelu:
                    nc.scalar.activation(
                        gslice,
                        ps[:],
                        mybir.ActivationFunctionType.Prelu,
                        bias=cvs[:, C_PB0:C_PB0 + 1],
                        scale=cvs[:, C_SC:C_SC + 1],
                        alpha=cvs[:, C_AL:C_AL + 1],
                    )
                else:
                    rt = rpool.tile([128, 512], F32, name="rt")
                    nc.scalar.activation(
                        rt[:],
                        ps[:],
                        mybir.ActivationFunctionType.Relu,
                        bias=cvs[:, C_RB:C_RB + 1],
                        scale=cvs[:, C_RS:C_RS + 1],
                    )
                    # g = a*scale*S + r   (r = (1-a)*relu(scale*S+pb0))
                    nc.vector.scalar_tensor_tensor(
                        gslice,
                        ps[:],
                        cvs[:, C_VS:C_VS + 1],
                        rt[:],
                        op0=mybir.AluOpType.mult,
                        op1=mybir.AluOpType.add,
                    )
            # final = (g + c) + x, in place over gt
            ccol = C_PB1 if use_prelu else C_VB
            nc.vector.scalar_tensor_tensor(
                gt[:],
                gt[:],
                cvs[:, ccol:ccol + 1],
                xt[:],
                op0=mybir.AluOpType.add,
                op1=mybir.AluOpType.add,
            )
            finv = gt[:].rearrange("p (s c) -> p s c", c=256)
            nc.gpsimd.dma_start(yd[b, :, r0:r0 + G:2, :], finv[0:64])
            nc.gpsimd.dma_start(yd[b, :, r0 + 1:r0 + G:2, :], finv[64:128])

        # software pipeline: loads/sign run 2 chunks ahead of the
        # matmul+epilogue consumer so the PE never starves at chunk edges.
        LOOKAHEAD = 2
        jobs = [(b, k) for b in range(B_per_core) for k in range(NCH)]
        xts = {}
        for idx, (b, k) in enumerate(jobs):
            xts[(b, k)] = load_sign_copy(b, k)
            if idx >= LOOKAHEAD:
                bb, kk = jobs[idx - LOOKAHEAD]
                conv_chunk(bb, kk, xts.pop((bb, kk)))
        for bb, kk in jobs[-LOOKAHEAD:]:
            conv_chunk(bb, kk, xts.pop((bb, kk)))

    return nc


def _host_prep(move0_bias, conv_weight, prelu_weight, pr_bias0, pr_bias1):
    """Pack weights into the 6 lhsT matrices + per-partition constant vectors."""
    w = np.asarray(conv_weight, dtype=np.float32)          # [O, I, 3, 3]
    sw = np.sign(w).astype(np.float32)                     # {-1, 0, 1}
    scale = np.mean(np.abs(w), axis=(1, 2, 3)).astype(np.float32)  # [O]
    a = np.asarray(prelu_weight, dtype=np.float32).reshape(64)
    pb0 = np.asarray(pr_bias0, dtype=np.float32).reshape(64)
    pb1 = np.asarray(pr_bias1, dtype=np.float32).reshape(64)
    b0 = np.asarray(move0_bias, dtype=np.float32).reshape(64)

    # lhsT[k, m] with k = pi*64 + ci, m = po*64 + co ->  sw[co, ci, kh, kw]
    # type-1: dh = [[0, -1], [1, 0]][pi][po]; type-2: only (pi0,po1)=+1,(pi1,po0)=-1
    wp = np.zeros((6, 128, 128), dtype=np.float32)
    swT = np.transpose(sw, (1, 0, 2, 3))  # [ci, co, kh, kw]
    for idw, dw in enumerate((-1, 0, 1)):
        kw = dw + 1
        # type-1
        wp[idw, 0:64, 0:64] = swT[:, :, 1, kw]      # even->even  dh=0
        wp[idw, 0:64, 64:128] = swT[:, :, 0, kw]    # even->odd   dh=-1 (kh=0)
        wp[idw, 64:128, 0:64] = swT[:, :, 2, kw]    # odd->even   dh=+1 (kh=2)
        wp[idw, 64:128, 64:128] = swT[:, :, 1, kw]  # odd->odd    dh=0
        # type-2
        wp[3 + idw, 0:64, 64:128] = swT[:, :, 2, kw]   # row 2i+2 -> out 2i+1, dh=+1
        wp[3 + idw, 64:128, 0:64] = swT[:, :, 0, kw]   # row 2i-1 -> out 2i,   dh=-1
    wp8 = wp.astype(NP_FP8)

    cv = np.zeros((128, NCOL), dtype=np.float32)
    for blk in range(2):
        s = slice(blk * 64, blk * 64 + 64)
        cv[s, C_B0] = b0
        cv[s, C_SC] = scale
        cv[s, C_PB0] = pb0
        cv[s, C_AL] = a
        cv[s, C_PB1] = pb1
        cv[s, C_RS] = (1.0 - a) * scale
        cv[s, C_RB] = (1.0 - a) * pb0
        cv[s, C_VS] = a * scale
        cv[s, C_VB] = a * pb0 + pb1
    return wp8, cv


_NC_CACHE: dict = {}


def _get_nc(key, *args):
    if key not in _NC_CACHE:
        _NC_CACHE[key] = _build(*args)
    return _NC_CACHE[key]


def kernel(x, move0_bias, conv_weight, prelu_weight, pr_bias0, pr_bias1):
    x = np.asarray(x, dtype=np.float32)
    B, C, H, W = x.shape
    NCORES = 8
    assert B % NCORES == 0
    Bc = B // NCORES
    G = 32
    use_prelu = os.environ.get("BBCU_NO_PRELU", "0") != "1"

    wp8, cv = _host_prep(move0_bias, conv_weight, prelu_weight, pr_bias0, pr_bias1)

    key = (Bc, H, W, C, G, use_prelu)
    nc = _get_nc(key, Bc, H, W, C, G, use_prelu)

    in_maps = [
        {"x": x[i * Bc:(i + 1) * Bc], "wp": wp8, "cv": cv} for i in range(NCORES)
    ]
    res = run_bass_kernel_spmd(nc, in_maps, core_ids=list(range(NCORES)))
    out = np.concatenate([res.results[i]["y"] for i in range(NCORES)], axis=0)
    return out.astype(np.float32)
